# revision 8
# baseline (speedup 1.0000x reference)
"""MemoryBank MoE-routing kernel for 8 Trainium2 NeuronCores.

Reference semantics (B=16, S=2048, D=1024, M=512, T=256, K=8):
    x0 = x[:, 0, :]                          # [B, D]
    scores = x0 @ memory_router              # [B, M]
    top_vals, top_idx = top_k(scores, 8)     # [B, K]
    w = softmax(top_vals)                    # [B, K]
    combined = sum_k w[b,k] * memory_tokens[top_idx[b,k]]   # [B, T, D]
    out = x;  out[:, 1:T+1, :] = combined

Sharding: data-parallel over batch (2 batches per core), memory_tokens and
memory_router replicated on every core.  No collectives.

Schedule (from per-engine DMA trace analysis): the 16 SDMA engines are the
bottleneck, and they stream at full rate only when ONE DMA queue is
draining at a time — two concurrent queues stretch every packet by an HBM
round-trip (measured 1.5-2x slowdown), so overlap never pays.  The kernel
therefore runs three strictly serial DMA phases:
  1. sync HWDGE: x0 + router (contiguous p-major layout, 16 KiB
     descriptors) + a pass-through chunk sized to exactly cover the
     routing-compute window (PE matmul + top-k), so the ring never idles;
  2. gpsimd SWDGE: 16 indirect expert gathers (1 MiB each, 8 KiB
     descriptors), batch 1 first then batch 0, FMA chains on DVE trailing
     each gather;
  3. sync HWDGE again: combined writes + the remaining pass-through.  The
     batch-0 write is first in program order and waits on the LAST FMA
     (batch 0 gathered last), which holds the independent pass-through
     tail back until the gather phase has drained.
Routing scores are computed REPLICATED on all 128 partitions (x0 column
broadcast to 128 stationary lanes) so top-k/softmax/row-id results are
usable per-partition with no broadcast step.
"""

import numpy as np

import concourse.bass as bass
import concourse.bacc as bacc
import concourse.mybir as mybir
from concourse import tile
from concourse.bass_utils import run_bass_kernel_spmd

N_CORES = 8
B, S, D = 16, 2048, 1024
M, T = 512, 256
K = 8
B_LOC = B // N_CORES  # batches per core
KT = D // 128         # contraction tiles for the router matmul

# pass-through rows T+1..S: PT0 (issued before the gathers) covers the
# routing-compute window (router load + cold-clock PE matmuls + top-k,
# ~20 us); the rest goes after the gather phase.
R0 = 832

F32 = mybir.dt.float32
U32 = mybir.dt.uint32


def build_program():
    nc = bacc.Bacc(
        "TRN2",
        target_bir_lowering=False,
        debug=False,
        num_devices=N_CORES,
    )

    x = nc.dram_tensor("x", [B_LOC, S, D], F32, kind="ExternalInput")
    mem = nc.dram_tensor("mem", [M, T, D], F32, kind="ExternalInput")
    router = nc.dram_tensor("router", [D, M], F32, kind="ExternalInput")
    out = nc.dram_tensor("out", [B_LOC, S, D], F32, kind="ExternalOutput")

    with tile.TileContext(nc) as tc:
        with (
            tc.tile_pool(name="sbuf", bufs=1) as sp,
            tc.tile_pool(name="gpool", bufs=10) as gp,
            tc.tile_pool(name="psum", bufs=1, space="PSUM") as pp,
        ):
            # ---- 1. routing inputs, first in the FIFO (critical path) ----
            # x0^T as (p, b, kt) with d = p*KT + kt (p-major), one DMA for
            # both batches.
            x0t = sp.tile([128, B_LOC * KT], F32)
            nc.sync.dma_start(
                out=x0t[:].rearrange("p (b kt) -> p b kt", b=B_LOC),
                in_=x[:, 0, :].rearrange("b (p kt) -> p b kt", kt=KT),
            )
            # router as (p, kt, m), same d = p*KT + kt split: partition p
            # holds rows p*KT..p*KT+KT-1 = 16 KiB contiguous.
            wt = sp.tile([128, KT * M], F32)
            nc.sync.dma_start(
                out=wt[:],
                in_=router[:, :].rearrange("(p kt) m -> p (kt m)", p=128),
            )

            # scheduler fence: keep the routing loads ahead of the
            # pass-through in the sync HWDGE FIFO
            tc.no_sync_barrier()

            # ---- 2. row-0 copy + pass-through chunk 0 ----
            nc.sync.dma_start(out=out[:, 0, :], in_=x[:, 0, :])
            # per-batch 2-D APs: the HWDGE engine split goes by the first
            # dimension, so a leading batch dim of 2 would land on only 2
            # of the 16 SDMA engines.
            for b in range(B_LOC):
                nc.sync.dma_start(
                    out=out[b, T + 1 : T + 1 + R0, :],
                    in_=x[b, T + 1 : T + 1 + R0, :],
                )
            tc.no_sync_barrier()

            # ---- 3. router scores, REPLICATED on all 128 partitions ----
            iota = sp.tile([128, 1], mybir.dt.int32)
            nc.gpsimd.iota(iota[:], pattern=[[0, 1]], base=0, channel_multiplier=1)
            iotaf = sp.tile([128, 1], F32)
            nc.vector.tensor_copy(out=iotaf[:], in_=iota[:])

            w_all = []
            ridu_all = []
            for b in range(B_LOC):
                scores_p = pp.tile([128, M], F32, name=f"scores{b}", tag=f"scores{b}")
                for kt in range(KT):
                    nc.tensor.matmul(
                        out=scores_p[:],
                        lhsT=x0t[:, b * KT + kt : b * KT + kt + 1].to_broadcast(
                            [128, 128]
                        ),
                        rhs=wt[:, kt * M : (kt + 1) * M],
                        start=(kt == 0),
                        stop=(kt == KT - 1),
                    )
                vals = sp.tile([128, K], F32, name=f"vals{b}", tag=f"vals{b}")
                nc.vector.max(out=vals[:], in_=scores_p[:])
                idx = sp.tile([128, K], U32, name=f"idx{b}", tag=f"idx{b}")
                nc.vector.max_index(out=idx[:], in_max=vals[:], in_values=scores_p[:])

                negmax = sp.tile([128, 1], F32, name=f"negmax{b}", tag=f"negmax{b}")
                nc.vector.tensor_scalar_mul(negmax[:], vals[:, 0:1], -1.0)
                ex = sp.tile([128, K], F32, name=f"ex{b}", tag=f"ex{b}")
                ssum = sp.tile([128, 1], F32, name=f"ssum{b}", tag=f"ssum{b}")
                nc.scalar.activation(
                    out=ex[:],
                    in_=vals[:],
                    func=mybir.ActivationFunctionType.Exp,
                    bias=negmax[:, 0:1],
                    scale=1.0,
                    accum_out=ssum[:, 0:1],
                )
                rec = sp.tile([128, 1], F32, name=f"rec{b}", tag=f"rec{b}")
                nc.vector.reciprocal(rec[:], ssum[:])
                w = sp.tile([128, K], F32, name=f"w{b}", tag=f"w{b}")
                nc.vector.tensor_scalar(
                    out=w[:],
                    in0=ex[:],
                    scalar1=rec[:, 0:1],
                    scalar2=None,
                    op0=mybir.AluOpType.mult,
                )
                w_all.append(w)

                # row indices into mem viewed [(m t2), (j d)]:
                # rid[p, k] = idx[b,k]*(T/2) + p   (two t-rows per row)
                idxf = sp.tile([128, K], F32, name=f"idxf{b}", tag=f"idxf{b}")
                nc.vector.tensor_copy(out=idxf[:], in_=idx[:])
                ridf = sp.tile([128, K], F32, name=f"ridf{b}", tag=f"ridf{b}")
                nc.vector.scalar_tensor_tensor(
                    out=ridf[:],
                    in0=idxf[:],
                    scalar=float(T // 2),
                    in1=iotaf[:, 0:1].to_broadcast([128, K]),
                    op0=mybir.AluOpType.mult,
                    op1=mybir.AluOpType.add,
                )
                ridu = sp.tile([128, K], U32, name=f"ridu{b}", tag=f"ridu{b}")
                nc.vector.tensor_copy(out=ridu[:], in_=ridf[:])
                ridu_all.append(ridu)

            # ---- 4. gathers (SWDGE), batch 1 first so batch 0's FMA chain
            # (whose write gates phase 3) completes last ----
            mem2 = mem[:, :, :].rearrange("m (t2 j) d -> (m t2) (j d)", j=2)
            cmbs = [
                sp.tile([128, 2 * D], F32, name=f"cmb{b}", tag=f"cmb{b}")
                for b in range(B_LOC)
            ]
            for b in reversed(range(B_LOC)):
                cmb = cmbs[b]
                for k in range(K):
                    g = gp.tile([128, 2 * D], F32, tag="g")
                    nc.gpsimd.indirect_dma_start(
                        out=g[:],
                        out_offset=None,
                        in_=mem2,
                        in_offset=bass.IndirectOffsetOnAxis(
                            ap=ridu_all[b][:, k : k + 1], axis=0
                        ),
                    )
                    if k == 0:
                        nc.vector.tensor_scalar_mul(cmb[:], g[:], w_all[b][:, k : k + 1])
                    else:
                        nc.vector.scalar_tensor_tensor(
                            out=cmb[:],
                            in0=g[:],
                            scalar=w_all[b][:, k : k + 1],
                            in1=cmb[:],
                            op0=mybir.AluOpType.mult,
                            op1=mybir.AluOpType.add,
                        )

            # ---- 5. combined writes + remaining pass-through.  write of
            # batch 0 (gathered LAST) leads: its semaphore wait blocks the
            # sync sequencer, holding the independent pass-through tail
            # back until the gather phase has drained. ----
            tc.no_sync_barrier()
            nc.sync.dma_start(
                out=out[0, 1 : T + 1, :].rearrange("(p j) d -> p j d", j=2),
                in_=cmbs[0][:].rearrange("p (j d) -> p j d", j=2),
            )
            nc.sync.dma_start(
                out=out[1, 1 : T + 1, :].rearrange("(p j) d -> p j d", j=2),
                in_=cmbs[1][:].rearrange("p (j d) -> p j d", j=2),
            )
            tc.no_sync_barrier()
            for b in range(B_LOC):
                nc.sync.dma_start(
                    out=out[b, T + 1 + R0 : S, :],
                    in_=x[b, T + 1 + R0 : S, :],
                )

    nc.compile()
    return nc


def kernel(x, memory_tokens, memory_router):
    nc = build_program()
    in_maps = [
        {
            "x": np.ascontiguousarray(x[c * B_LOC : (c + 1) * B_LOC]),
            "mem": memory_tokens,
            "router": memory_router,
        }
        for c in range(N_CORES)
    ]
    res = run_bass_kernel_spmd(nc, in_maps, list(range(N_CORES)))
    return np.concatenate(
        [res.results[c]["out"] for c in range(N_CORES)], axis=0
    )


# revision 9
# speedup vs baseline: 1.0501x; 1.0501x over previous
"""MemoryBank MoE-routing kernel for 8 Trainium2 NeuronCores.

Reference semantics (B=16, S=2048, D=1024, M=512, T=256, K=8):
    x0 = x[:, 0, :]                          # [B, D]
    scores = x0 @ memory_router              # [B, M]
    top_vals, top_idx = top_k(scores, 8)     # [B, K]
    w = softmax(top_vals)                    # [B, K]
    combined = sum_k w[b,k] * memory_tokens[top_idx[b,k]]   # [B, T, D]
    out = x;  out[:, 1:T+1, :] = combined

Sharding: data-parallel over batch (2 batches per core), memory_tokens and
memory_router replicated on every core.  No collectives.

Schedule (from per-engine DMA trace analysis):
  * sync HWDGE ring: x0 + router first (scheduler fence pins them ahead),
    then the entire pass-through copy as one continuous stream.  The
    stream covers the routing-compute window and afterwards overlaps the
    gather phase (measured mixed rate ~19+28 GB/s per engine, better than
    serial phasing which leaves engines idle at phase seams).
  * pool SWDGE ring: 16 indirect expert gathers.  Routing is computed for
    batch 1 FIRST so its gathers can start the moment its top-k lands
    (~12 us earlier than computing batch 0 first).  The combined writes
    ride the same pool ring behind the last gather: Q7 waits on the FMA
    semaphores, the ring never round-robins against the sync ring's
    pass-through at packet granularity for them.
  * DVE FMA chains trail each gather; per-batch accumulators.
Routing scores are computed REPLICATED on all 128 partitions (x0 column
broadcast to 128 stationary lanes) so top-k/softmax/row-id results are
usable per-partition with no broadcast step.
"""

import numpy as np

import concourse.bass as bass
import concourse.bacc as bacc
import concourse.mybir as mybir
from concourse import tile
from concourse.bass_utils import run_bass_kernel_spmd

N_CORES = 8
B, S, D = 16, 2048, 1024
M, T = 512, 256
K = 8
B_LOC = B // N_CORES  # batches per core
KT = D // 128         # contraction tiles for the router matmul

F32 = mybir.dt.float32
U32 = mybir.dt.uint32

# batch order: routing + gathers for batch 1 first, so gathers start as
# early as possible while batch 0 routes.
BORDER = (1, 0)


def build_program():
    nc = bacc.Bacc(
        "TRN2",
        target_bir_lowering=False,
        debug=False,
        num_devices=N_CORES,
    )

    x = nc.dram_tensor("x", [B_LOC, S, D], F32, kind="ExternalInput")
    mem = nc.dram_tensor("mem", [M, T, D], F32, kind="ExternalInput")
    router = nc.dram_tensor("router", [D, M], F32, kind="ExternalInput")
    out = nc.dram_tensor("out", [B_LOC, S, D], F32, kind="ExternalOutput")

    with tile.TileContext(nc) as tc:
        with (
            tc.tile_pool(name="sbuf", bufs=1) as sp,
            tc.tile_pool(name="gpool", bufs=12) as gp,
            tc.tile_pool(name="psum", bufs=1, space="PSUM") as pp,
        ):
            # ---- 1. routing inputs, first in the sync FIFO ----
            x0t = sp.tile([128, B_LOC * KT], F32)
            nc.sync.dma_start(
                out=x0t[:].rearrange("p (b kt) -> p b kt", b=B_LOC),
                in_=x[:, 0, :].rearrange("b (p kt) -> p b kt", kt=KT),
            )
            # router as (p, kt, m) with d = p*KT + kt: partition p holds
            # rows p*KT..p*KT+KT-1 = 16 KiB contiguous.
            wt = sp.tile([128, KT * M], F32)
            nc.sync.dma_start(
                out=wt[:],
                in_=router[:, :].rearrange("(p kt) m -> p (kt m)", p=128),
            )
            nc.sync.dma_start(out=out[:, 0, :], in_=x[:, 0, :])
            # fence: keep the routing loads ahead of the pass-through
            tc.no_sync_barrier()

            # ---- 2. full pass-through stream (per-batch 2-D APs: the
            # engine split goes by the first dimension) ----
            for b in range(B_LOC):
                nc.sync.dma_start(
                    out=out[b, T + 1 : S, :],
                    in_=x[b, T + 1 : S, :],
                )

            # ---- 3. router scores, REPLICATED on all 128 partitions,
            # batch 1 first ----
            iota = sp.tile([128, 1], mybir.dt.int32)
            nc.gpsimd.iota(iota[:], pattern=[[0, 1]], base=0, channel_multiplier=1)
            iotaf = sp.tile([128, 1], F32)
            nc.vector.tensor_copy(out=iotaf[:], in_=iota[:])

            w_all = {}
            ridu_all = {}
            for b in BORDER:
                scores_p = pp.tile([128, M], F32, name=f"scores{b}", tag=f"scores{b}")
                for kt in range(KT):
                    nc.tensor.matmul(
                        out=scores_p[:],
                        lhsT=x0t[:, b * KT + kt : b * KT + kt + 1].to_broadcast(
                            [128, 128]
                        ),
                        rhs=wt[:, kt * M : (kt + 1) * M],
                        start=(kt == 0),
                        stop=(kt == KT - 1),
                    )
                vals = sp.tile([128, K], F32, name=f"vals{b}", tag=f"vals{b}")
                nc.vector.max(out=vals[:], in_=scores_p[:])
                idx = sp.tile([128, K], U32, name=f"idx{b}", tag=f"idx{b}")
                nc.vector.max_index(out=idx[:], in_max=vals[:], in_values=scores_p[:])

                negmax = sp.tile([128, 1], F32, name=f"negmax{b}", tag=f"negmax{b}")
                nc.vector.tensor_scalar_mul(negmax[:], vals[:, 0:1], -1.0)
                ex = sp.tile([128, K], F32, name=f"ex{b}", tag=f"ex{b}")
                ssum = sp.tile([128, 1], F32, name=f"ssum{b}", tag=f"ssum{b}")
                nc.scalar.activation(
                    out=ex[:],
                    in_=vals[:],
                    func=mybir.ActivationFunctionType.Exp,
                    bias=negmax[:, 0:1],
                    scale=1.0,
                    accum_out=ssum[:, 0:1],
                )
                rec = sp.tile([128, 1], F32, name=f"rec{b}", tag=f"rec{b}")
                nc.vector.reciprocal(rec[:], ssum[:])
                w = sp.tile([128, K], F32, name=f"w{b}", tag=f"w{b}")
                nc.vector.tensor_scalar(
                    out=w[:],
                    in0=ex[:],
                    scalar1=rec[:, 0:1],
                    scalar2=None,
                    op0=mybir.AluOpType.mult,
                )
                w_all[b] = w

                # rid[p, k] = idx[b,k]*(T/2) + p for mem viewed
                # [(m t2), (j d)] (two t-rows per gathered row)
                idxf = sp.tile([128, K], F32, name=f"idxf{b}", tag=f"idxf{b}")
                nc.vector.tensor_copy(out=idxf[:], in_=idx[:])
                ridf = sp.tile([128, K], F32, name=f"ridf{b}", tag=f"ridf{b}")
                nc.vector.scalar_tensor_tensor(
                    out=ridf[:],
                    in0=idxf[:],
                    scalar=float(T // 2),
                    in1=iotaf[:, 0:1].to_broadcast([128, K]),
                    op0=mybir.AluOpType.mult,
                    op1=mybir.AluOpType.add,
                )
                ridu = sp.tile([128, K], U32, name=f"ridu{b}", tag=f"ridu{b}")
                nc.vector.tensor_copy(out=ridu[:], in_=ridf[:])
                ridu_all[b] = ridu

            # ---- 4. gathers (pool SWDGE ring), batch 1 then batch 0,
            # FMA chains trailing on DVE ----
            mem2 = mem[:, :, :].rearrange("m (t2 j) d -> (m t2) (j d)", j=2)
            cmbs = {
                b: sp.tile([128, 2 * D], F32, name=f"cmb{b}", tag=f"cmb{b}")
                for b in BORDER
            }
            for b in BORDER:
                cmb = cmbs[b]
                for k in range(K):
                    g = gp.tile([128, 2 * D], F32, tag="g")
                    nc.gpsimd.indirect_dma_start(
                        out=g[:],
                        out_offset=None,
                        in_=mem2,
                        in_offset=bass.IndirectOffsetOnAxis(
                            ap=ridu_all[b][:, k : k + 1], axis=0
                        ),
                    )
                    if k == 0:
                        nc.vector.tensor_scalar_mul(cmb[:], g[:], w_all[b][:, k : k + 1])
                    else:
                        nc.vector.scalar_tensor_tensor(
                            out=cmb[:],
                            in0=g[:],
                            scalar=w_all[b][:, k : k + 1],
                            in1=cmb[:],
                            op0=mybir.AluOpType.mult,
                            op1=mybir.AluOpType.add,
                        )

            # ---- 5. combined writes on the SAME pool ring, behind the
            # last gather (Q7 waits the FMA semaphores in FIFO order) ----
            tc.no_sync_barrier()
            for b in BORDER:
                nc.gpsimd.dma_start(
                    out=out[b, 1 : T + 1, :].rearrange("(p j) d -> p j d", j=2),
                    in_=cmbs[b][:].rearrange("p (j d) -> p j d", j=2),
                )

    nc.compile()
    return nc


def kernel(x, memory_tokens, memory_router):
    nc = build_program()
    in_maps = [
        {
            "x": np.ascontiguousarray(x[c * B_LOC : (c + 1) * B_LOC]),
            "mem": memory_tokens,
            "router": memory_router,
        }
        for c in range(N_CORES)
    ]
    res = run_bass_kernel_spmd(nc, in_maps, list(range(N_CORES)))
    return np.concatenate(
        [res.results[c]["out"] for c in range(N_CORES)], axis=0
    )
